# revision 1
# baseline (speedup 1.0000x reference)
"""Multi-head causal attention (B=2, T=2048, C=1024, H=16, S=64) on 8 TRN2 cores.

Sharding: core i handles batch b = i//4 and head group g = i%4 (4 heads each).
Each core computes a partial output projection (its heads' contribution to the
full [T, C] output); the host sums the 4 partials per batch and adds the bias.

Per-core dataflow (all layouts chosen so no on-chip transposes are needed;
bf16 matmuls with f32 PSUM accumulation throughout):
  qT/kT   [S, T]  = W.T @ x.T        (head-pair packed in the 128 partitions)
  v       [T, S]                     (bf16 stored, ones column appended for d)
  attT    [Tk, Tq] = kT-tile.T @ qT  (K=64; exact-causal tiles only)
  p       = exp(0.125 * attT)        (ACT, bf16 out; diagonal masked via 0/1 mul)
  yT|d    [S+1, Tq] = [v|1].T @ p    (row 64 = softmax denominator)
  yT_norm = yT * (1/d)               (reciprocal + partition_broadcast + mul)
  out     [T, C] partial = yT.T @ WpT (head-pair stacked contraction)
"""

import os
import math
import numpy as np
import ml_dtypes

import concourse.bacc as bacc
import concourse.mybir as mybir
import concourse.tile as tile
from concourse.bass_utils import run_bass_kernel_spmd

F32 = mybir.dt.float32
F32R = mybir.dt.float32r
BF16 = mybir.dt.bfloat16

B, T, C, H, S = 2, 2048, 1024, 16, 64
HPC = 4          # heads per core
N_CORES = 8
NC_T = T // 128  # 16 t-tiles of 128

# attT storage offsets: tile tk spans tq in [128*tk, 2048)
SPAN = [T - 128 * tk for tk in range(NC_T)]
OFF = [0] * NC_T
for _tk in range(1, NC_T):
    OFF[_tk] = OFF[_tk - 1] + SPAN[_tk - 1]
ATT_W = OFF[-1] + SPAN[-1]  # 17408

_cached_nc = None
last_results = None  # BassKernelResults of the most recent run (for test harness)


def _build():
    nc = bacc.Bacc("TRN2", target_bir_lowering=False)

    # bf16 QKV inputs, pre-chunked on host so each DMA is one big contiguous-
    # per-partition transfer (128 rows x 2-8KB): c-chunk c of wq[hp] lives at
    # cols [128c:128c+128], of wv at cols [256c:256c+256].
    xT_d = nc.dram_tensor("xT", [C, T], BF16, kind="ExternalInput")
    wq_d = nc.dram_tensor("wq", [2, 128, 8 * 128], BF16, kind="ExternalInput")
    wk_d = nc.dram_tensor("wk", [2, 128, 8 * 128], BF16, kind="ExternalInput")
    wv_d = nc.dram_tensor("wv", [128, 8 * 256], BF16, kind="ExternalInput")
    wpT_d = nc.dram_tensor("wpT", [2, 128, C], BF16, kind="ExternalInput")
    mask_d = nc.dram_tensor("mask", [128, 128], BF16, kind="ExternalInput")
    out_d = nc.dram_tensor("out", [T, C], BF16, kind="ExternalOutput")

    with tile.TileContext(nc) as tc:
        with (
            tc.tile_pool(name="const", bufs=1) as constp,
            tc.tile_pool(name="qkT", bufs=1) as qkp,
            tc.tile_pool(name="vsb", bufs=1) as vp,
            tc.tile_pool(name="yT", bufs=1) as ytp,
            tc.tile_pool(name="attT", bufs=1) as attp,
            tc.tile_pool(name="yps", bufs=2, space="PSUM") as yps,
            tc.tile_pool(name="sm", bufs=2) as smp,
        ):
            # persistent tiles
            mask_sb = constp.tile([128, 128], BF16, name="mask_sb")
            nc.sync.dma_start(mask_sb[:], mask_d[:])

            qT2 = [qkp.tile([128, T], BF16, name=f"qT2_{hp}") for hp in range(2)]
            kT2 = [qkp.tile([128, T], BF16, name=f"kT2_{hp}") for hp in range(2)]
            # v tiles: [128, 4*65] bf16; head h in cols 65h..65h+63, col 65h+64 = 1
            v_sb = [vp.tile([128, 4 * 65], BF16, name=f"v{t}") for t in range(NC_T)]
            for t in range(NC_T):
                ones_ap = v_sb[t].rearrange("p (h c) -> p h c", h=4)[:, :, 64]
                nc.vector.memset(ones_ap, 1.0)
            yT_all = [ytp.tile([128, T], BF16, name=f"yTa{hp}") for hp in range(2)]
            att_buf = [
                attp.tile([128, ATT_W], BF16, name=f"attb{i}") for i in range(3)
            ]
            BUF_OF = [0, 1, 2, 0]  # head -> attT buffer

            def emit_scores_tk(h, tk):
                hp, half = h // 2, h % 2
                r0 = 64 * half
                ab = att_buf[BUF_OF[h]]
                krow = kT2[hp][r0 : r0 + 64, :]
                qrow = qT2[hp][r0 : r0 + 64, :]
                span = SPAN[tk]
                kt = krow[:, 128 * tk : 128 * tk + 128]
                for part in range(math.ceil(span / 1024)):
                    pspan = min(1024, span - 1024 * part)
                    pt = sps.tile([128, 1024], F32, name="sps_t", tag="s")
                    for mmi in range(math.ceil(pspan / 512)):
                        n = min(512, pspan - 512 * mmi)
                        tq0 = 128 * tk + 1024 * part + 512 * mmi
                        nc.tensor.matmul(
                            pt[:, 512 * mmi : 512 * mmi + n],
                            kt,
                            qrow[:, tq0 : tq0 + n],
                            start=True,
                            stop=True,
                        )
                    dst = ab[
                        :, OFF[tk] + 1024 * part : OFF[tk] + 1024 * part + pspan
                    ]
                    nc.scalar.activation(
                        dst,
                        pt[:, 0:pspan],
                        mybir.ActivationFunctionType.Exp,
                        scale=0.125,
                    )
                # mask the diagonal block (first 128 cols of this tk tile)
                diag = ab[:, OFF[tk] : OFF[tk] + 128]
                nc.vector.tensor_mul(diag, diag, mask_sb[:])

            def emit_y_window(h, j):
                hp, half = h // 2, h % 2
                ab = att_buf[BUF_OF[h]]
                yp = yps.tile([65, 512], F32, name="yps_t", tag="y")
                tk_hi = min(NC_T - 1, 4 * j + 3)
                for tk in range(tk_hi + 1):
                    if 128 * tk <= 512 * j:
                        n = 512
                        outc = 0
                        ac = OFF[tk] + 512 * j - 128 * tk
                    else:
                        n = 512 * (j + 1) - 128 * tk
                        outc = 128 * tk - 512 * j
                        ac = OFF[tk]
                    nc.tensor.matmul(
                        yp[:, outc : outc + n],
                        v_sb[tk][:, 65 * h : 65 * h + 65],
                        ab[:, ac : ac + n],
                        start=(tk == 0),
                        stop=(tk == tk_hi),
                        skip_group_check=True,
                    )
                # normalize: yT_norm = yT * (1/d), d in psum row 64
                rec = smp.tile([1, 512], F32, name="rec")
                nc.vector.reciprocal(rec[:], yp[64:65, :])
                bc = smp.tile([64, 512], F32, name="bc")
                nc.gpsimd.partition_broadcast(bc[:], rec[:])
                dst = yT_all[hp][
                    64 * half : 64 * half + 64, 512 * j : 512 * j + 512
                ]
                if half == 0:
                    nc.vector.tensor_mul(dst, yp[0:64, :], bc[:])
                else:
                    stg = smp.tile([64, 512], BF16, name="stg")
                    nc.vector.tensor_mul(stg[:], yp[0:64, :], bc[:])
                    # SWDGE queue: keeps the partition shift off the HWDGE
                    # queue that carries the big input/output transfers.
                    nc.gpsimd.dma_start(dst, stg[:])

            # ---- scores/QKV scope: sps closes after phase E ----
            wpT_sb = [
                constp.tile([128, C], BF16, name=f"wpT{hp}") for hp in range(2)
            ]
            with (
                tc.tile_pool(name="sps", bufs=2, space="PSUM") as sps,
            ):
              with (
                tc.tile_pool(name="xw", bufs=1) as xw,
                tc.tile_pool(name="mmps", bufs=2, space="PSUM") as mmps,
              ):
                # x first (the QK c-loop consumes chunks in order), weights
                # adjacent to first use; all transfers are 128 x 2-8KB rows.
                wq_sb = [
                    xw.tile([128, 1024], BF16, name=f"wq{hp}") for hp in range(2)
                ]
                wk_sb = [
                    xw.tile([128, 1024], BF16, name=f"wk{hp}") for hp in range(2)
                ]
                wv_sb = xw.tile([128, 2048], BF16, name="wv")
                xT_sb = [xw.tile([128, T], BF16, name=f"xT{c}") for c in range(8)]
                nc.sync.dma_start(wq_sb[0][:], wq_d[0])
                # half-major loads: the first two QK groups only need
                # cols 0-1023 of every chunk, so they can start after ~2MB
                # of the 4MB x transfer instead of all of it.
                for half in range(2):
                    for c in range(8):
                        nc.sync.dma_start(
                            xT_sb[c][:, 1024 * half : 1024 * half + 1024],
                            xT_d[
                                128 * c : 128 * c + 128,
                                1024 * half : 1024 * half + 1024,
                            ],
                        )
                nc.sync.dma_start(wk_sb[0][:], wk_d[0])
                nc.sync.dma_start(wv_sb[:], wv_d[:])
                nc.sync.dma_start(wq_sb[1][:], wq_d[1])
                nc.sync.dma_start(wk_sb[1][:], wk_d[1])

                def emit_qk_group(hp, kind, tq):
                    w_sb = wq_sb if kind == 0 else wk_sb
                    dst = qT2[hp] if kind == 0 else kT2[hp]
                    pt = mmps.tile([128, 512], F32, name="qkps", tag="qk")
                    for c in range(8):
                        nc.tensor.matmul(
                            pt[:],
                            w_sb[hp][:, 128 * c : 128 * c + 128],
                            xT_sb[c][:, 512 * tq : 512 * tq + 512],
                            start=(c == 0),
                            stop=(c == 7),
                        )
                    nc.vector.tensor_copy(dst[:, 512 * tq : 512 * tq + 512], pt[:])

                def emit_v_t(t):
                    pv = mmps.tile([128, 256], F32, name="vps", tag="qk")
                    for c in range(8):
                        nc.tensor.matmul(
                            pv[:],
                            xT_sb[c][:, 128 * t : 128 * t + 128],
                            wv_sb[:, 256 * c : 256 * c + 256],
                            start=(c == 0),
                            stop=(c == 7),
                        )
                    nc.vector.tensor_copy(
                        v_sb[t].rearrange("p (h c) -> p h c", h=4)[:, :, 0:64],
                        pv[:].rearrange("p (h c) -> p h c", h=4),
                    )

                # PE warm-up: dummy matmuls on the mask tile while the
                # first input DMAs are in flight (HAM clock-gate warm-up).
                warm = sps.tile([128, 1024], F32, name="warm", tag="s")
                for i in range(24):
                    nc.tensor.matmul(
                        warm[:, 0:128],
                        mask_sb[:],
                        mask_sb[:],
                        start=True,
                        stop=True,
                    )
                # Phase A: q projections for head-pair 0.
                for tq in range(4):
                    emit_qk_group(0, 0, tq)
                for hp in range(2):
                    nc.gpsimd.dma_start(wpT_sb[hp][:], wpT_d[hp])
                # Phase B: k(hp0) + scores h0 + q(hp1) filler.
                for g in range(4):
                    emit_qk_group(0, 1, g)
                    for tk in range(4 * g, 4 * g + 4):
                        emit_scores_tk(0, tk)
                    emit_qk_group(1, 0, g)
                # Phase C: k(hp1) + scores h1 + first half of v.
                for g in range(4):
                    emit_qk_group(1, 1, g)
                    for tk in range(4 * g, 4 * g + 4):
                        emit_scores_tk(1, tk)
                    emit_v_t(2 * g)
                    emit_v_t(2 * g + 1)
                # Phase D: scores h2 + second half of v + y(h0) windows.
                for g in range(4):
                    for tk in range(4 * g, 4 * g + 4):
                        emit_scores_tk(2, tk)
                    emit_v_t(8 + 2 * g)
                    emit_v_t(9 + 2 * g)
                    emit_y_window(0, g)

              # Phase E: scores h3 + y(h1) + y(h2) windows (sps still open).
              for g in range(4):
                  for tk in range(4 * g, 4 * g + 4):
                      emit_scores_tk(3, tk)
                  emit_y_window(1, g)
                  emit_y_window(2, g)

            # ---- projection (sps closed: pps gets its 4 banks) ----
            with (
                tc.tile_pool(name="pps", bufs=4, space="PSUM") as pps,
                tc.tile_pool(name="outs", bufs=8) as outs,
            ):
                def emit_proj_pair(t0):
                    # hp0 halves first: they depend only on earlier heads, so
                    # they hide the y(h3) normalize chain of the current batch.
                    pps_t = {}
                    for t in (t0, t0 + 1):
                        for n in range(2):
                            pp = pps.tile([128, 512], F32, name="pp", tag="p")
                            pps_t[t, n] = pp
                            nc.tensor.matmul(
                                pp[:],
                                yT_all[0][:, 128 * t : 128 * t + 128],
                                wpT_sb[0][:, 512 * n : 512 * n + 512],
                                start=True,
                                stop=False,
                                skip_group_check=True,
                            )
                    for t in (t0, t0 + 1):
                        for n in range(2):
                            pp = pps_t[t, n]
                            nc.tensor.matmul(
                                pp[:],
                                yT_all[1][:, 128 * t : 128 * t + 128],
                                wpT_sb[1][:, 512 * n : 512 * n + 512],
                                start=False,
                                stop=True,
                                skip_group_check=True,
                            )
                            ot = outs.tile([128, 512], BF16, name="ot")
                            # alternate engines: ACT is idle once exp is done
                            if n == 0:
                                nc.vector.tensor_copy(ot[:], pp[:])
                            else:
                                nc.scalar.copy(ot[:], pp[:])
                            # final batch: split across both DMA queues
                            eng = nc.gpsimd if (t >= 14 and n == 1) else nc.sync
                            eng.dma_start(
                                out_d[
                                    128 * t : 128 * t + 128,
                                    512 * n : 512 * n + 512,
                                ],
                                ot[:],
                            )

                # Phase F: y(h3) windows one batch ahead of their
                # projection, so each normalize chain hides under the
                # previous batch's proj matmuls.
                emit_y_window(3, 0)
                emit_y_window(3, 1)
                for j in range(4):
                    emit_proj_pair(4 * j)
                    if j < 2:
                        emit_y_window(3, j + 2)
                    emit_proj_pair(4 * j + 2)

    nc.finalize()
    return nc


def _get_nc():
    global _cached_nc
    if _cached_nc is None:
        _cached_nc = _build()
    return _cached_nc


def kernel(x, Wq, Wk, Wv, Wp, bp):
    global last_results
    x = np.asarray(x, dtype=np.float32)
    Wq = np.asarray(Wq, dtype=np.float32)
    Wk = np.asarray(Wk, dtype=np.float32)
    Wv = np.asarray(Wv, dtype=np.float32)
    Wp = np.asarray(Wp, dtype=np.float32)
    bp = np.asarray(bp, dtype=np.float32)

    WpT = np.ascontiguousarray(Wp.T)  # [C_in(features), C_out]
    mask01 = np.triu(np.ones((128, 128), dtype=np.float32)).astype(ml_dtypes.bfloat16)

    def chunked(w):
        # [C, m] -> [128, 8*m]: c-chunk c at cols [m*c : m*(c+1)]
        m = w.shape[1]
        return np.ascontiguousarray(
            w.reshape(8, 128, m).transpose(1, 0, 2).reshape(128, 8 * m)
        ).astype(ml_dtypes.bfloat16)

    xT_by_batch = [
        np.ascontiguousarray(x[b].T).astype(ml_dtypes.bfloat16) for b in range(B)
    ]
    in_maps = []
    for core in range(N_CORES):
        b, g = core // 4, core % 4
        h0 = HPC * g
        wq_p = np.stack(
            [chunked(np.concatenate([Wq[h0 + 2 * hp], Wq[h0 + 2 * hp + 1]], axis=1))
             for hp in range(2)]
        )  # [2, 128, 1024] bf16
        wk_p = np.stack(
            [chunked(np.concatenate([Wk[h0 + 2 * hp], Wk[h0 + 2 * hp + 1]], axis=1))
             for hp in range(2)]
        )
        wv_p = chunked(
            np.concatenate([Wv[h0 + j] for j in range(HPC)], axis=1)
        )  # [128, 2048] bf16
        wpT_p = np.ascontiguousarray(
            WpT[256 * g : 256 * (g + 1)].reshape(2, 128, C)
        ).astype(ml_dtypes.bfloat16)
        in_maps.append(
            {
                "xT": xT_by_batch[b],
                "wq": wq_p,
                "wk": wk_p,
                "wv": wv_p,
                "wpT": wpT_p,
                "mask": mask01,
            }
        )

    nc = _get_nc()
    kwargs = {}
    if os.environ.get("KERNEL_TRACE", "0") == "1":
        kwargs = dict(trace=True, trace_cores=list(range(N_CORES)),
                      stitch_traces=True)
    try:
        res = run_bass_kernel_spmd(
            nc, in_maps, core_ids=list(range(N_CORES)), **kwargs
        )
    except ModuleNotFoundError:
        # tracing unavailable in this environment; run untraced
        res = run_bass_kernel_spmd(nc, in_maps, core_ids=list(range(N_CORES)))
    last_results = res

    out = np.zeros((B, T, C), dtype=np.float32)
    for core in range(N_CORES):
        b = core // 4
        out[b] += res.results[core]["out"].astype(np.float32)
    out += bp[None, None, :]
    return out

